# revision 32
# baseline (speedup 1.0000x reference)
"""Trainium2 Bass kernel for nn_KGPathReasoner.

8-core SPMD, data-parallel over the entity-pair dimension P.
Each core handles 256 pairs = 2560 paths; embedding tables + weights replicated.

Device layout is feature-major (features on SBUF partitions, tokens on the free
dim) throughout:
  - tokens are sorted by path length (descending) on host, so LSTM step t only
    runs over the live prefix; the per-(chunk, step) live widths AND the
    per-(chunk, step) selection ranges (tokens with len == t+1, a contiguous
    range in sorted order) are computed from the actual path_lens and baked
    into the program (programs are cached per (widths, ranges) signature;
    widths/ranges are cross-core maxima, per-core exactness is restored by
    small 0/1 masks shipped as data),
  - embedding tables are pre-cast to bf16 on host; rows are gathered
    token-major via indirect DMA and transposed feature-major on the tensor
    engine (single-pass bf16 matmul against identity),
  - all matmul operands are bf16 (PSUM accumulation stays fp32); weights are
    pre-folded on host:
      M_cat = [kg_proj_w.T @ w_ih.T[:512] ; kg_proj_w.T @ w_ih.T[512:]]
    so the kg projection never materializes,
  - the LSTM runs over 512-token chunks with gates accumulated in PSUM
    (x-side + h-side matmuls) over the live width only, sigmoid/tanh on the
    scalar engine with the fused per-partition bias, cell updates on the
    vector engine (c stays fp32, h is written bf16 for the next step's
    matmul),
  - h(len-1) selection is a narrow predicated copy over the baked range
    (len == t+1 tokens form a contiguous sorted range); masks come from DRAM
    per core; the selected embeddings are transposed token-major per chunk,
    round-tripped through a DRAM scratch buffer, and gathered back in the
    original token order (the scratch output buffer is zero-donated, so len-0
    paths read back as zero),
  - attention uses a block-diagonal trick: groups of 8 pairs = 80 tokens, all
    4 heads' [80x80] scores packed into one PSUM bank, exp on ACT, then a
    multiplicative 0/1 block-diag mask on the vector engine, column-sum via
    ones-matmul, batched reciprocal, and a partition-broadcast of the
    reciprocals via a K=1 ones matmul on the tensor engine (PSUM), then a
    v.T @ attn matmul giving ctx feature-major directly,
  - cell-state math runs in bf16 on the vector engine (2x DVE rate; rel_fro
    stays ~4e-3 vs the 2e-2 gate); engine assignment balances ACT vs DVE
    (q/k biases + v/den copies on ACT, casts/cell updates/ctx scaling on DVE),
  - q/k per 512-chunk, v per 80-group, and the score/softmax/ctx core per
    4-group batch are interleaved with the unsort read-back by token
    availability, so the tensor engine streams through the phase transition,
  - attn_out + mean-over-paths + path_proj fold into a single matmul
    (attention rows sum to 1, everything after ctx is linear):
      out = Wf @ sum_k(ctx') + bf,  Wf = wpp @ wao / K,
      bf = wpp @ wao @ bv + wpp @ bao + bpp.
"""

import numpy as np
import ml_dtypes

BF16 = ml_dtypes.bfloat16

NCORES = 8
P, KP, L = 2048, 10, 3
E, H = 256, 512
N_ENT, N_REL = 10000, 200
NHEADS, DH = 4, 128
P_LOC = P // NCORES           # 256 pairs per core
N_LOC = P_LOC * KP            # 2560 paths per core
CH = 512                      # LSTM token chunk
NCH = N_LOC // CH             # 5
NTG = N_LOC // 128            # 20 token groups of 128
AG = 80                       # attention group = 8 pairs * 10 paths
PAIRS_G = AG // KP            # 8
GB = 4                        # attention groups per reciprocal batch

_PROGS = {}


def _plan(path_lens):
    """Baked plan from actual lens: per-(chunk, step) live width (multiple of
    16, cross-core max) and selection range (cross-core union)."""
    S = np.zeros((NCORES, L + 2), np.int64)
    for core in range(NCORES):
        lens = path_lens[core * P_LOC:(core + 1) * P_LOC].reshape(-1)
        for l in range(L + 2):
            S[core, l] = int((lens >= l).sum())
    Smax = S.max(0)
    Smin = S.min(0)
    steps = []   # (c, t, n16)
    sels = []    # (c, t, a, b) absolute sorted positions
    for c in range(NCH):
        for t in range(L):
            n = min(CH, int(Smax[t + 1]) - c * CH)
            if n <= 0:
                continue
            n16 = min(CH, -(-n // 16) * 16)
            steps.append((c, t, n16))
            a = max(c * CH, int(Smin[t + 2]))
            b = min((c + 1) * CH, int(Smax[t + 1]))
            if b > a:
                sels.append((c, t, a, b))
    return tuple(steps), tuple(sels)


def _build_program(steps, sels):
    import concourse.bass as bass
    import concourse.mybir as mybir
    import concourse.tile as tile
    from concourse import bacc

    f32 = mybir.dt.float32
    bf16 = mybir.dt.bfloat16
    i32 = mybir.dt.int32
    AF = mybir.ActivationFunctionType
    OP = mybir.AluOpType

    step_map = {}
    for c, t, n16 in steps:
        step_map.setdefault(c, []).append((t, n16))
    sel_map = {}
    woff = 0
    for c, t, a, b in sels:
        sel_map[(c, t)] = (a, b, woff)
        woff += b - a
    WSEL = woff
    NJ = sum(-(-n16 // 128) for _, _, n16 in steps)   # gather-index columns
    n_proc = len(step_map)                            # chunks with steps

    nc = bacc.Bacc()

    # ---- DRAM parameters (per core) ----
    ent_table = nc.declare_dram_parameter("ent_table_bf", [N_ENT, E], bf16, isOutput=False)
    rel_table = nc.declare_dram_parameter("rel_table_bf", [N_REL, E], bf16, isOutput=False)
    rel_idx_d = nc.declare_dram_parameter("rel_idx_p", [128, NJ], i32, isOutput=False)
    ent_idx_d = nc.declare_dram_parameter("ent_idx_p", [128, NJ], i32, isOutput=False)
    gb_d = nc.declare_dram_parameter("gb_idx", [128, NTG], i32, isOutput=False)
    selmask_d = nc.declare_dram_parameter("selmask", [128, max(WSEL, 4)], i32, isOutput=False)
    mcat_d = nc.declare_dram_parameter("mcat_t", [2 * E, 4 * H], bf16, isOutput=False)
    whh_d = nc.declare_dram_parameter("whh_t", [H, 4 * H], bf16, isOutput=False)
    gbias_d = nc.declare_dram_parameter("gate_bias", [128, 16], f32, isOutput=False)
    wq_d = nc.declare_dram_parameter("wq_t", [H, H], bf16, isOutput=False)
    wk_d = nc.declare_dram_parameter("wk_t", [H, H], bf16, isOutput=False)
    wv_d = nc.declare_dram_parameter("wv_t", [H, H], bf16, isOutput=False)
    bq_d = nc.declare_dram_parameter("bq_p", [128, 4], f32, isOutput=False)
    bk_d = nc.declare_dram_parameter("bk_p", [128, 4], f32, isOutput=False)
    # fused tail: out = Wf @ sum_k(ctx) + bf, Wf = wpp @ wao / K,
    # bf = wpp @ wao @ bv + wpp @ bao + bpp (attention rows sum to 1)
    wf_d = nc.declare_dram_parameter("wf_t", [H, H], bf16, isOutput=False)
    bf_d = nc.declare_dram_parameter("bf_p", [128, 4], f32, isOutput=False)
    mmul_d = nc.declare_dram_parameter("mask_mult", [128, NHEADS * AG], bf16, isOutput=False)
    # token-major path-embedding scratch (sorted order); declared as an output
    # so PJRT donates a zeroed buffer -> unwritten (len-0 / unprocessed) rows
    # read back as zero
    hscr_d = nc.declare_dram_parameter("h_scr", [N_LOC, H], bf16, isOutput=True)
    out_d = nc.declare_dram_parameter("out", [H, P_LOC], f32, isOutput=True)

    with tile.TileContext(nc) as tc:
        # ---------- persistent pool (spans both phases) ----------
        with tc.tile_pool(name="persist", bufs=1) as pp:
            # h_sel: selected h, feature-major, SORTED token order
            h_sel = [pp.tile([128, n_proc * CH], bf16, name=f"h_sel{i}") for i in range(4)]
            # h_org: path_emb, feature-major, ORIGINAL token order
            h_org = [pp.tile([128, N_LOC], bf16, name=f"h_org{i}") for i in range(4)]

            ones_t = pp.tile([128, H], bf16, name="ones_t")
            ones_f = pp.tile([1, 128], f32, name="ones_f")

            def emit_deferred_memsets():
                # deferred so the first chunk-step's casts lead the DVE queue
                for hs in h_sel:
                    nc.vector.memset(hs[:], 0.0)
                nc.vector.memset(ones_t[:], 1.0)
                nc.vector.memset(ones_f[:], 1.0)

            ident = pp.tile([128, 128], bf16, name="ident")
            from concourse.masks import make_identity
            make_identity(nc, ident[:])

            mmul_sb = pp.tile([128, NHEADS * AG], bf16, name="mmul_sb")
            nc.sync.dma_start(out=mmul_sb[:], in_=mmul_d[:, :])

            gb_sb = pp.tile([128, NTG], i32, name="gb_sb")
            nc.sync.dma_start(out=gb_sb[:], in_=gb_d[:, :])

            # attention weights prefetched at program start (DMA idle in LSTM)
            wq_sb = [pp.tile([128, H], bf16, name=f"wq{i}") for i in range(4)]
            wk_sb = [pp.tile([128, H], bf16, name=f"wk{i}") for i in range(4)]
            wv_sb = [pp.tile([128, H], bf16, name=f"wv{i}") for i in range(4)]
            wf_sb = [pp.tile([128, H], bf16, name=f"wf{i}") for i in range(4)]
            bq_sb = pp.tile([128, 4], f32, name="bq_sb")
            bk_sb = pp.tile([128, 4], f32, name="bk_sb")
            bf_sb = pp.tile([128, 4], f32, name="bf_sb")

            # ---------- phase 1: encode + LSTM (sorted order) ----------
            with tc.tile_pool(name="lw", bufs=1) as lw, \
                 tc.tile_pool(name="lstm_sb", bufs=2) as ls, \
                 tc.tile_pool(name="gath", bufs=40) as gp, \
                 tc.tile_pool(name="xcat", bufs=12) as xp, \
                 tc.tile_pool(name="sig", bufs=8) as sg, \
                 tc.tile_pool(name="xtp", bufs=2, space="PSUM") as xtp, \
                 tc.tile_pool(name="gpsum", bufs=4, space="PSUM") as gpsum:

                mcat_sb = [lw.tile([128, 4 * H], bf16, name=f"mcat{i}") for i in range(4)]
                whh_sb = [lw.tile([128, 4 * H], bf16, name=f"whh{i}") for i in range(4)]
                ridx_sb = lw.tile([128, NJ], i32, name="ridx_sb")
                eidx_sb = lw.tile([128, NJ], i32, name="eidx_sb")
                gb_psb = lw.tile([128, 16], f32, name="gb_psb")
                selmask_sb = lw.tile([128, max(WSEL, 4)], i32, name="selmask_sb")
                # critical-path first: idx + first mcat tiles gate step (0,0);
                # everything else is deferred into the chunk loop so the first
                # gathers' DMA legs do not queue behind bulk weight loads
                nc.sync.dma_start(out=ridx_sb[:], in_=rel_idx_d[:, :])
                nc.sync.dma_start(out=eidx_sb[:], in_=ent_idx_d[:, :])
                nc.sync.dma_start(out=gb_psb[:], in_=gbias_d[:, :])
                for i in range(4):
                    nc.sync.dma_start(out=mcat_sb[i][:], in_=mcat_d[i * 128:(i + 1) * 128, :])

                def emit_deferred_dmas_a(anchor):
                    for i in range(4):
                        nc.sync.dma_start(out=whh_sb[i][:], in_=whh_d[i * 128:(i + 1) * 128, :])
                    nc.sync.dma_start(out=selmask_sb[:], in_=selmask_d[:, :])

                def emit_deferred_dmas_b(anchor):
                    # anchored on chunk-1 compute: keeps 4MB of attention
                    # weights off the DMA queues while the ramp's gather legs
                    # stream (first needed ~150us later, loads in ~15us)
                    for i in range(4):
                        for sb, dd in ((wq_sb, wq_d), (wk_sb, wk_d), (wv_sb, wv_d), (wf_sb, wf_d)):
                            d = nc.sync.dma_start(out=sb[i][:], in_=dd[i * 128:(i + 1) * 128, :])
                            tile.add_dep_helper(d.ins, anchor.ins, reason="defer attn load")
                    nc.sync.dma_start(out=bq_sb[:], in_=bq_d[:, :])
                    nc.sync.dma_start(out=bk_sb[:], in_=bk_d[:, :])
                    nc.sync.dma_start(out=bf_sb[:], in_=bf_d[:, :])

                jctr = 0
                wr_insts = []
                pending_unsort = []

                def emit_unsort_a(c):
                    # unsort part A: chunk c's h_sel -> token-major h_scr rows
                    # (PE transpose + DMA out); deferred one chunk so it does
                    # not fence the next chunk's PE stream
                    for g in range(CH // 128):
                        s0 = c * CH + g * 128
                        tp = gpsum.tile([128, CH], f32, name="tp", tag="gpsum", space="PSUM")
                        for ft in range(4):
                            nc.tensor.matmul(
                                out=tp[:, ft * 128:(ft + 1) * 128],
                                lhsT=h_sel[ft][:, s0:s0 + 128],
                                rhs=ident[:], start=True, stop=True)
                        htm = xp.tile([128, CH], bf16, name="htm", tag="xcat")
                        nc.vector.tensor_copy(out=htm[:], in_=tp[:])
                        wr = nc.sync.dma_start(out=hscr_d[s0:s0 + 128, :], in_=htm[:])
                        wr_insts.append(wr)

                for c in sorted(step_map):
                    h_prev = [None] * 4
                    c_prev = [None] * 4
                    first_step = True
                    for t, n16 in step_map[c]:
                        ng = -(-n16 // 128)
                        # gathers (token-major [128, 256] bf16 per 128-token group)
                        grels, gents = [], []
                        for g in range(ng):
                            j = jctr + g
                            grel = gp.tile([128, E], bf16, name="grel", tag="gath")
                            nc.gpsimd.indirect_dma_start(
                                out=grel[:], out_offset=None, in_=rel_table[:, :],
                                in_offset=bass.IndirectOffsetOnAxis(
                                    ap=ridx_sb[:, j:j + 1], axis=0))
                            grels.append(grel)
                        for g in range(ng):
                            j = jctr + g
                            gent = gp.tile([128, E], bf16, name="gent", tag="gath")
                            nc.gpsimd.indirect_dma_start(
                                out=gent[:], out_offset=None, in_=ent_table[:, :],
                                in_offset=bass.IndirectOffsetOnAxis(
                                    ap=eidx_sb[:, j:j + 1], axis=0))
                            gents.append(gent)
                        gts = list(zip(grels, gents))
                        jctr += ng

                        # transpose to feature-major xc [4][128, n16] on PE
                        gw = ng * 128
                        xt_rel = xtp.tile([128, 2 * CH], f32, name="xt_rel", tag="xt", space="PSUM")
                        xt_ent = xtp.tile([128, 2 * CH], f32, name="xt_ent", tag="xt", space="PSUM")
                        for g in range(ng):
                            for half in range(2):
                                nc.tensor.matmul(
                                    out=xt_rel[:, half * gw + g * 128:half * gw + g * 128 + 128],
                                    lhsT=grels[g][:, half * 128:(half + 1) * 128],
                                    rhs=ident[:], start=True, stop=True)
                        for g in range(ng):
                            for half in range(2):
                                nc.tensor.matmul(
                                    out=xt_ent[:, half * gw + g * 128:half * gw + g * 128 + 128],
                                    lhsT=gents[g][:, half * 128:(half + 1) * 128],
                                    rhs=ident[:], start=True, stop=True)
                        xc = []
                        for i in range(4):
                            xi = xp.tile([128, CH], bf16, name="xi", tag="xcat")
                            srct = (xt_rel, xt_ent)[i // 2]
                            if i % 2 == 0:
                                nc.scalar.activation(
                                    out=xi[:, :n16],
                                    in_=srct[:, (i % 2) * gw:(i % 2) * gw + n16], func=AF.Copy)
                            else:
                                nc.vector.tensor_copy(
                                    out=xi[:, :n16], in_=srct[:, (i % 2) * gw:(i % 2) * gw + n16])
                            xc.append(xi)

                        # gates: one PSUM bank per gate [i, f, g, o]
                        h_new = [None] * 4
                        c_new = [None] * 4
                        for ft in range(4):
                            gps = {}
                            for gi, m in enumerate((ft, 4 + ft, 8 + ft, 12 + ft)):
                                if t == 0 and gi == 1:
                                    continue  # forget gate unused when c==0
                                gt = gpsum.tile([128, CH], f32, name="gt", tag="gpsum", space="PSUM")
                                gps[gi] = gt
                                for kt in range(4):
                                    nc.tensor.matmul(
                                        out=gt[:, :n16], lhsT=mcat_sb[kt][:, m * 128:(m + 1) * 128],
                                        rhs=xc[kt][:, :n16], start=(kt == 0), stop=(t == 0 and kt == 3))
                                if t > 0:
                                    for kt in range(4):
                                        nc.tensor.matmul(
                                            out=gt[:, :n16], lhsT=whh_sb[kt][:, m * 128:(m + 1) * 128],
                                            rhs=h_prev[kt][:, :n16], start=False, stop=(kt == 3))
                            si = sg.tile([128, CH], bf16, name="si", tag="sig")
                            tg = sg.tile([128, CH], bf16, name="tg", tag="sig")
                            so = sg.tile([128, CH], bf16, name="so", tag="sig")
                            nc.scalar.activation(out=si[:, :n16], in_=gps[0][:, :n16],
                                                 func=AF.Sigmoid, bias=gb_psb[:, ft:ft + 1])
                            nc.scalar.activation(out=tg[:, :n16], in_=gps[2][:, :n16],
                                                 func=AF.Tanh, bias=gb_psb[:, 8 + ft:9 + ft])
                            nc.scalar.activation(out=so[:, :n16], in_=gps[3][:, :n16],
                                                 func=AF.Sigmoid, bias=gb_psb[:, 12 + ft:13 + ft])
                            cn = ls.tile([128, CH], bf16, name="cn", tag=f"c{ft}", bufs=2)
                            if t == 0:
                                nc.vector.tensor_tensor(out=cn[:, :n16], in0=si[:, :n16],
                                                        in1=tg[:, :n16], op=OP.mult)
                            else:
                                sf = sg.tile([128, CH], bf16, name="sf", tag="sig")
                                nc.scalar.activation(out=sf[:, :n16], in_=gps[1][:, :n16],
                                                     func=AF.Sigmoid, bias=gb_psb[:, 4 + ft:5 + ft])
                                tmp = sg.tile([128, CH], bf16, name="tmp", tag="sig")
                                nc.vector.tensor_tensor(out=cn[:, :n16], in0=sf[:, :n16],
                                                        in1=c_prev[ft][:, :n16], op=OP.mult)
                                nc.vector.tensor_tensor(out=tmp[:, :n16], in0=si[:, :n16],
                                                        in1=tg[:, :n16], op=OP.mult)
                                nc.vector.tensor_tensor(out=cn[:, :n16], in0=cn[:, :n16],
                                                        in1=tmp[:, :n16], op=OP.add)
                            tc_t = sg.tile([128, CH], bf16, name="tc_t", tag="sig")
                            nc.scalar.activation(out=tc_t[:, :n16], in_=cn[:, :n16], func=AF.Tanh)
                            hn = ls.tile([128, CH], bf16, name="hn", tag=f"h{ft}", bufs=2)
                            hn_ins = nc.vector.tensor_tensor(out=hn[:, :n16], in0=so[:, :n16],
                                                             in1=tc_t[:, :n16], op=OP.mult)
                            if (c, t) in sel_map:
                                a, b, off = sel_map[(c, t)]
                                w = b - a
                                nc.vector.copy_predicated(
                                    out=h_sel[ft][:, a:b],
                                    mask=selmask_sb[:, off:off + w],
                                    data=hn[:, a - c * CH:b - c * CH])
                            h_new[ft] = hn
                            c_new[ft] = cn
                        h_prev = h_new
                        c_prev = c_new
                        if first_step:
                            first_step = False
                            if not wr_insts and not pending_unsort:
                                emit_deferred_dmas_a(hn_ins)
                                emit_deferred_memsets()
                            elif len(wr_insts) == 0 and len(pending_unsort) == 1:
                                emit_deferred_dmas_b(hn_ins)
                            while pending_unsort:
                                emit_unsort_a(pending_unsort.pop(0))
                    pending_unsort.append(c)
                while pending_unsort:
                    emit_unsort_a(pending_unsort.pop(0))

            # ---------- phase 2: attention (original order, dense) ----------
            NGG = N_LOC // AG  # 32 independent pair-groups
            with tc.tile_pool(name="asml", bufs=8) as asml, \
                 tc.tile_pool(name="aw2", bufs=1) as aw:

                agg_sb = [aw.tile([128, P_LOC], f32, name=f"agg{i}") for i in range(4)]
                agg_bf = [aw.tile([128, P_LOC], bf16, name=f"aggb{i}") for i in range(4)]

                # Pipelined attention: q/k per 512-chunk, v per 80-group, and
                # the score/softmax/ctx core per 4-group batch are interleaved
                # by data availability so PE streams while the unsort
                # read-back is still gathering later chunks.
                q_all = [aw.tile([128, N_LOC], bf16, name=f"q_all{i}") for i in range(4)]
                k_all = [aw.tile([128, N_LOC], bf16, name=f"k_all{i}") for i in range(4)]
                ctx_all = [aw.tile([128, N_LOC], bf16, name=f"ctx{i}") for i in range(4)]
                v_tm = [aw.tile([128, H], bf16, name=f"vsb{gg}") for gg in range(NGG)]

                # PSUM budget (8 banks): qk/v ring "aps2" x3, score ring
                # "aps3" x2, ctx/broadcast/final ring "aps4" x3
                qkv_psum = tc.tile_pool(name="qkvp", bufs=2, space="PSUM")
                aps2 = qkv_psum.__enter__()
                core_psum4 = tc.tile_pool(name="corep4", bufs=2, space="PSUM")
                aps4 = core_psum4.__enter__()

                def emit_qk(cc):
                    ss = cc * CH
                    for m in range(4):
                        qps = aps2.tile([128, CH], f32, name="qps", tag="aps2", space="PSUM")
                        kps = aps2.tile([128, CH], f32, name="kps", tag="aps2", space="PSUM")
                        for kt in range(4):
                            nc.tensor.matmul(
                                out=qps[:], lhsT=wq_sb[kt][:, m * 128:(m + 1) * 128],
                                rhs=h_org[kt][:, ss:ss + CH], start=(kt == 0), stop=(kt == 3))
                            nc.tensor.matmul(
                                out=kps[:], lhsT=wk_sb[kt][:, m * 128:(m + 1) * 128],
                                rhs=h_org[kt][:, ss:ss + CH], start=(kt == 0), stop=(kt == 3))
                        nc.scalar.activation(out=q_all[m][:, ss:ss + CH], in_=qps[:],
                                             func=AF.Identity, bias=bq_sb[:, m:m + 1])
                        nc.scalar.activation(out=k_all[m][:, ss:ss + CH], in_=kps[:],
                                             func=AF.Identity, bias=bk_sb[:, m:m + 1])

                def emit_v(gg):
                    so_ = gg * AG
                    vp = aps2.tile([128, H], f32, name="vp", tag="aps2", space="PSUM")
                    for kt in range(4):
                        nc.tensor.matmul(
                            out=vp[:AG, :], lhsT=h_org[kt][:, so_:so_ + AG],
                            rhs=wv_sb[kt][:], start=(kt == 0), stop=(kt == 3))
                    nc.scalar.activation(out=v_tm[gg][:AG, :], in_=vp[:AG, :], func=AF.Copy)

                def emit_batch_front(gb):
                    # 4 groups: scores -> exp -> mask -> colsum; batched DVE
                    # reciprocal via DMA reshape (front half of the batch
                    # pipeline; the back half runs after the NEXT batch's
                    # scores so PE never stalls on the reciprocal DMA latency)
                    exbs = []
                    den_flat = asml.tile([1, GB * NHEADS * AG], f32, name="den_flat", tag="denf", bufs=2)
                    for gi in range(GB):
                        gg = gb * GB + gi
                        o = gg * AG
                        exb = asml.tile([128, NHEADS * AG], bf16, name="exb", tag="exb", bufs=12)
                        exbs.append(exb)
                        # all 4 heads' scores in one PSUM bank (start only on the
                        # first head: flags=0 writes overwrite untouched columns)
                        sc = aps2.tile([128, NHEADS * AG], f32, name="sc", tag="aps3", space="PSUM", bufs=2)
                        for hh in range(NHEADS):
                            nc.tensor.matmul(
                                out=sc[:AG, hh * AG:(hh + 1) * AG], lhsT=k_all[hh][:, o:o + AG],
                                rhs=q_all[hh][:, o:o + AG], start=(hh == 0), stop=(hh == 3))
                        # exp then multiplicative 0/1 block-diag mask on DVE
                        nc.scalar.activation(out=exb[:AG, :], in_=sc[:AG, :],
                                             func=AF.Exp, scale=float(1.0 / np.sqrt(DH)))
                        nc.vector.tensor_tensor(out=exb[:AG, :], in0=exb[:AG, :],
                                                in1=mmul_sb[:AG, :], op=OP.mult)
                        cs = aps2.tile([1, NHEADS * AG], f32, name="cs", tag="aps3", space="PSUM", bufs=2)
                        nc.tensor.matmul(out=cs[:1, :], lhsT=ones_t[:AG, :1],
                                         rhs=exb[:AG, :], start=True, stop=True)
                        nc.scalar.activation(
                            out=den_flat[0:1, gi * NHEADS * AG:(gi + 1) * NHEADS * AG],
                            in_=cs[:1, :], func=AF.Copy)
                    # reshape 1x1280 -> 16x80, one parallel reciprocal (the
                    # [1, N] form is lane-serial and 3x slower), cast bf16,
                    # reshape back
                    den_p = asml.tile([GB * NHEADS, AG], f32, name="den_p", tag="denp", bufs=2)
                    nc.sync.dma_start(out=den_p[:, :], in_=den_flat[0:1, :])
                    rcp_p = asml.tile([GB * NHEADS, AG], f32, name="rcp_p", tag="rcpp", bufs=2)
                    nc.vector.reciprocal(out=rcp_p[:], in_=den_p[:])
                    rcp_b = asml.tile([GB * NHEADS, AG], bf16, name="rcp_b", tag="rcpb", bufs=2)
                    nc.vector.tensor_copy(out=rcp_b[:], in_=rcp_p[:])
                    rcp_flat = asml.tile([1, GB * NHEADS * AG], bf16, name="rcp_flat", tag="rcpf", bufs=2)
                    nc.sync.dma_start(out=rcp_flat[0:1, :], in_=rcp_b[:, :])
                    return exbs, rcp_flat

                def emit_batch_back(gb, exbs, rcp_flat):
                    for gi in range(GB):
                        gg = gb * GB + gi
                        o = gg * AG
                        exb = exbs[gi]
                        # partition-broadcast of reciprocals via K=1 ones matmul
                        rbp = aps4.tile([128, NHEADS * AG], f32, name="rbp", tag="aps4", space="PSUM")
                        nc.tensor.matmul(
                            out=rbp[:], lhsT=ones_t[0:1, :128],
                            rhs=rcp_flat[0:1, gi * NHEADS * AG:(gi + 1) * NHEADS * AG],
                            start=True, stop=True)
                        rb = asml.tile([128, NHEADS * AG], bf16, name="rb", tag="rb", bufs=4)
                        nc.vector.tensor_copy(out=rb[:], in_=rbp[:])
                        for hh in range(NHEADS):
                            cxp = aps4.tile([128, AG], f32, name="cxp", tag="aps4", space="PSUM")
                            nc.tensor.matmul(
                                out=cxp[:, :], lhsT=v_tm[gg][:AG, hh * 128:(hh + 1) * 128],
                                rhs=exb[:AG, hh * AG:(hh + 1) * AG], start=True, stop=True)
                            nc.vector.scalar_tensor_tensor(
                                out=ctx_all[hh][:, o:o + AG], in0=cxp[:, :],
                                scalar=1.0, in1=rb[:, hh * AG:(hh + 1) * AG],
                                op0=OP.mult, op1=OP.mult)

                    # sum this batch's ctx over each pair's K paths
                    # (tail matmul is folded); 320 tokens = 32 pairs per batch
                    ob = gb * GB * AG
                    pb = gb * (GB * AG // KP)
                    for m in range(4):
                        nc.vector.reduce_sum(
                            out=agg_sb[m][:, pb:pb + GB * AG // KP],
                            in_=ctx_all[m][:, ob:ob + GB * AG].rearrange("p (a k) -> p a k", k=KP),
                            axis=mybir.AxisListType.X)

                # interleave by chunk availability: after chunk cc's h_org
                # lands, emit its q/k, then every v-group and core-batch whose
                # token span is fully covered
                def emit_unsort(cc):
                    # unsort part B for this chunk: gather back in ORIGINAL
                    # token order + transpose feature-major
                    for g2 in range(4 * cc, 4 * cc + 4):
                        hg = asml.tile([128, H], bf16, name="hg", tag="hg", bufs=12)
                        rd = nc.gpsimd.indirect_dma_start(
                            out=hg[:], out_offset=None, in_=hscr_d[:, :],
                            in_offset=bass.IndirectOffsetOnAxis(
                                ap=gb_sb[:, g2:g2 + 1], axis=0))
                        for wr in wr_insts:
                            tile.add_dep_helper(rd.ins, wr.ins, reason="h_scr RAW round-trip")
                        tp2 = aps2.tile([128, CH], f32, name="tp2", tag="tp2", space="PSUM", bufs=2)
                        for ft in range(4):
                            nc.tensor.matmul(
                                out=tp2[:, ft * 128:(ft + 1) * 128],
                                lhsT=hg[:, ft * 128:(ft + 1) * 128],
                                rhs=ident[:], start=True, stop=True)
                        for ft in range(4):
                            nc.vector.tensor_copy(
                                out=h_org[ft][:, g2 * 128:(g2 + 1) * 128],
                                in_=tp2[:, ft * 128:(ft + 1) * 128])

                v_done = 0
                b_done = 0
                pend = None
                for cc in range(NCH):
                    emit_unsort(cc)
                    emit_qk(cc)
                    end = (cc + 1) * CH
                    while v_done < NGG and (v_done + 1) * AG <= end:
                        emit_v(v_done)
                        v_done += 1
                    while b_done < NGG // GB and (b_done + 1) * GB * AG <= end:
                        st = emit_batch_front(b_done)
                        if pend is not None:
                            emit_batch_back(*pend)
                        pend = (b_done, *st)
                        b_done += 1
                while b_done < NGG // GB:
                    st = emit_batch_front(b_done)
                    if pend is not None:
                        emit_batch_back(*pend)
                    pend = (b_done, *st)
                    b_done += 1
                if pend is not None:
                    emit_batch_back(*pend)

                for m in range(4):
                    nc.vector.tensor_copy(out=agg_bf[m][:], in_=agg_sb[m][:])

                # fused attn_out + mean + path_proj: out = Wf @ aggS + bf
                for m in range(4):
                    pps = aps4.tile([128, P_LOC], f32, name="pps", tag="aps4", space="PSUM")
                    for kt in range(4):
                        nc.tensor.matmul(
                            out=pps[:], lhsT=wf_sb[kt][:, m * 128:(m + 1) * 128],
                            rhs=agg_bf[kt][:], start=(kt == 0), stop=(kt == 3))
                    osb = asml.tile([128, P_LOC], f32, name="osb", tag="osb", bufs=4)
                    nc.scalar.activation(out=osb[:], in_=pps[:], func=AF.Identity,
                                         bias=bf_sb[:, m:m + 1])
                    nc.sync.dma_start(out=out_d[m * 128:(m + 1) * 128, :], in_=osb[:])
                core_psum4.__exit__(None, None, None)
                qkv_psum.__exit__(None, None, None)

    nc.compile()
    return nc


def _prep_host(inputs, steps, sels):
    """Fold weights and lay out indices host-side. Returns (shared, per_core)."""
    f = np.float32
    kg_proj_w = np.asarray(inputs["kg_proj_w"], f)      # [H, E]
    kg_proj_b = np.asarray(inputs["kg_proj_b"], f)      # [H]
    w_ih = np.asarray(inputs["w_ih"], f)                # [4H, 2H]
    w_hh = np.asarray(inputs["w_hh"], f)                # [4H, H]
    b_ih = np.asarray(inputs["b_ih"], f)
    b_hh = np.asarray(inputs["b_hh"], f)
    attn_in_w = np.asarray(inputs["attn_in_w"], f)      # [3H, H]
    attn_in_b = np.asarray(inputs["attn_in_b"], f)
    attn_out_w = np.asarray(inputs["attn_out_w"], f)    # [H, H]
    attn_out_b = np.asarray(inputs["attn_out_b"], f)
    path_proj_w = np.asarray(inputs["path_proj_w"], f)  # [H, H]
    path_proj_b = np.asarray(inputs["path_proj_b"], f)

    W1 = w_ih[:, :H].T                                   # [H, 4H] (rel_p part)
    W2 = w_ih[:, H:].T                                   # [H, 4H] (ent_p part)
    M_r = kg_proj_w.T @ W1                               # [E, 4H]
    M_e = kg_proj_w.T @ W2                               # [E, 4H]
    mcat_t = np.ascontiguousarray(np.concatenate([M_r, M_e], axis=0))  # [2E, 4H]
    gate_bias = kg_proj_b @ W1 + kg_proj_b @ W2 + b_ih + b_hh          # [4H]

    bd = np.zeros((128, NHEADS * AG), f)
    for hh in range(NHEADS):
        for pg in range(PAIRS_G):
            bd[pg * KP:(pg + 1) * KP, hh * AG + pg * KP:hh * AG + (pg + 1) * KP] = 1.0

    # fused tail: out = Wf @ sum_k(ctx) + bf  (attention rows sum to 1, so bv
    # passes through; mean + attn_out + path_proj are all linear)
    bv = attn_in_b[2 * H:]
    ppw_aow = path_proj_w @ attn_out_w                   # [H, H]
    wf_eff = ppw_aow / KP
    bf_vec = ppw_aow @ bv + path_proj_w @ attn_out_b + path_proj_b

    shared = {
        "ent_table_bf": np.ascontiguousarray(np.asarray(inputs["ent_table"], f).astype(BF16)),
        "rel_table_bf": np.ascontiguousarray(np.asarray(inputs["rel_table"], f).astype(BF16)),
        "mcat_t": np.ascontiguousarray(mcat_t.astype(BF16)),
        "whh_t": np.ascontiguousarray(w_hh.T.astype(BF16)),
        "gate_bias": np.ascontiguousarray(gate_bias.reshape(16, 128).T),
        "wq_t": np.ascontiguousarray(attn_in_w[:H].T.astype(BF16)),
        "wk_t": np.ascontiguousarray(attn_in_w[H:2 * H].T.astype(BF16)),
        "wv_t": np.ascontiguousarray(attn_in_w[2 * H:].T.astype(BF16)),
        "bq_p": np.ascontiguousarray(attn_in_b[:H].reshape(4, 128).T),
        "bk_p": np.ascontiguousarray(attn_in_b[H:2 * H].reshape(4, 128).T),
        "wf_t": np.ascontiguousarray(wf_eff.T.astype(BF16)),
        "bf_p": np.ascontiguousarray(bf_vec.reshape(4, 128).T),
        "mask_mult": np.ascontiguousarray(bd.astype(BF16)),
    }

    rel_idx = np.asarray(inputs["rel_idx"])              # [P, K, L] int32
    ent_idx = np.asarray(inputs["ent_idx"])
    path_lens = np.asarray(inputs["path_lens"])          # [P, K] int32

    NJ = sum(-(-n16 // 128) for _, _, n16 in steps)
    WSEL = sum(b - a for _, _, a, b in sels)
    per_core = []
    for core in range(NCORES):
        sl = slice(core * P_LOC, (core + 1) * P_LOC)
        lens = path_lens[sl].reshape(N_LOC)
        perm = np.argsort(-lens, kind="stable")
        inv = np.empty(N_LOC, np.int64)
        inv[perm] = np.arange(N_LOC)
        ri = rel_idx[sl].reshape(N_LOC, L)[perm]
        ei = ent_idx[sl].reshape(N_LOC, L)[perm]
        ls_srt = lens[perm]
        rj = np.empty((128, NJ), np.int32)
        ej = np.empty((128, NJ), np.int32)
        jj = 0
        for c, t, n16 in steps:
            for g in range(-(-n16 // 128)):
                s0 = c * CH + g * 128
                rj[:, jj] = ri[s0:s0 + 128, t]
                ej[:, jj] = ei[s0:s0 + 128, t]
                jj += 1
        sm = np.zeros((128, max(WSEL, 4)), np.int32)
        off = 0
        for c, t, a, b in sels:
            sm[:, off:off + (b - a)] = (ls_srt[a:b] == t + 1).astype(np.int32)[None, :]
            off += b - a
        gbi = np.ascontiguousarray(inv.reshape(NTG, 128).T.astype(np.int32))
        per_core.append({"rel_idx_p": rj, "ent_idx_p": ej, "selmask": sm,
                         "gb_idx": gbi})
    return shared, per_core


def _run(inputs, trace=False):
    from concourse.bass_utils import run_bass_kernel_spmd
    steps, sels = _plan(np.asarray(inputs["path_lens"]))
    key = (steps, sels)
    if key not in _PROGS:
        _PROGS[key] = _build_program(steps, sels)
    prog = _PROGS[key]
    shared, per_core = _prep_host(inputs, steps, sels)
    in_maps = [{**shared, **pc} for pc in per_core]
    res = run_bass_kernel_spmd(prog, in_maps, list(range(NCORES)), trace=trace)
    out = np.concatenate([np.ascontiguousarray(r["out"].T) for r in res.results], axis=0)
    return out, res


def kernel(**inputs):
    out, _ = _run(inputs, trace=False)
    return out
